# revision 12
# baseline (speedup 1.0000x reference)
"""Trainium2 Bass kernel for BatteryMoEFlattenIntraCycleMoELayer.

out[b] = sum_{e in top2(b)} gate[b,e] * (x[b] @ W_e.T + bias_e),  cast to bf16

Strategy: data-parallel over B across 8 cores (16 samples/core).
Per core, on device:
  - gating (softmax numerator -> mask -> top-2 -> renormalize) on tiny [16,8]
  - top-2 dispatch as 16 matmuls per sample: x-tile stationary (M=L=100),
    expert weight tile moving, selected at runtime via a PE-register AP offset
  - gates folded in by pre-scaling x on DVE; bias folded in as an extra
    ones-row of x against a bias-row of W
All matmul data is bf16 (fp32 matmul runs at 1/4 rate on PE); accumulation is
fp32 in PSUM; output cast to bf16 on eviction.
"""

import numpy as np
import ml_dtypes
from contextlib import ExitStack

import concourse.bass as bass
import concourse.bacc as bacc
import concourse.mybir as mybir
import concourse.tile as tile
from concourse.bass_utils import run_bass_kernel_spmd

# problem shape (hardcoded per contract)
B, L, C, CURVE = 128, 100, 3, 300
F = C * CURVE            # 900
E, D, TOPK = 8, 512, 2
EPS = 1e-9

NCORES = 8
BL = B // NCORES         # 16 samples per core
KT = 8                   # contraction tiles of 128 (900+bias row padded to 1024)
FP = KT * 128            # 1024
WAVE = 8                 # samples in flight (one PSUM bank each)
XCH = 4                  # samples per x-DMA chunk

BF16 = mybir.dt.bfloat16
F32 = mybir.dt.float32
I32 = mybir.dt.int32
U32 = mybir.dt.uint32

_BF = ml_dtypes.bfloat16

_NC_CACHE = {}


def _emit_body(nc, tc, ctx, xh, wh, lg, mk, out, R=""):
    PE = mybir.EngineType.PE

    gp = ctx.enter_context(tc.tile_pool(name=f"{R}gating", bufs=1))
    wp = ctx.enter_context(tc.tile_pool(name=f"{R}wpool", bufs=1))
    xp = ctx.enter_context(tc.tile_pool(name=f"{R}xpool", bufs=XCH))
    xsp = ctx.enter_context(tc.tile_pool(name=f"{R}xspool", bufs=2 * BL))
    pp = ctx.enter_context(tc.tile_pool(name=f"{R}psum", bufs=WAVE, space="PSUM"))
    op = ctx.enter_context(tc.tile_pool(name=f"{R}outp", bufs=WAVE))

    # ---- gating inputs first (tiny, fast) on the SP HWDGE ring
    lg_sb = gp.tile([BL, E], F32, name=f"{R}lg_sb")
    mk_sb = gp.tile([BL, E], I32, name=f"{R}mk_sb")
    nc.sync.dma_start(lg_sb, lg[:, :])
    nc.sync.dma_start(mk_sb, mk[:, :])

    # ---- bulk loads on the SP HWDGE ring: w0 + wave-1 x first, then the
    # remaining W chunks paced to the k-major burst rate, wave-2 x last
    w_t = [wp.tile([128, E * D], BF16, tag=f"w{k}", name=f"{R}w{k}")
           for k in range(KT)]
    x_t = [xp.tile([128, XCH * KT * L], BF16, tag="xch", name=f"{R}xch{c}")
           for c in range(BL // XCH)]

    def _load_x(c):
        nc.sync.dma_start(x_t[c], xh[:, c * XCH * KT * L:(c + 1) * XCH * KT * L])

    _load_x(0)
    nc.sync.dma_start(w_t[0], wh[0])
    _load_x(1)
    for k in range(1, KT):
        nc.sync.dma_start(w_t[k], wh[k])
    _load_x(2)
    _load_x(3)

    # ---- gating math: samples on partitions, experts on free dim.
    # softmax denominator cancels in the top-2 renorm, so use the
    # numerator p = exp(lg - rowmax); eps is scaled by Z to match.
    rowmax = gp.tile([BL, 1], F32, name=f"{R}rowmax")
    nc.vector.tensor_reduce(rowmax, lg_sb, axis=mybir.AxisListType.X,
                            op=mybir.AluOpType.max)
    negmax = gp.tile([BL, 1], F32, name=f"{R}negmax")
    nc.vector.tensor_scalar_mul(negmax, rowmax, -1.0)
    p_t = gp.tile([BL, E], F32, name=f"{R}p_t")
    z_t = gp.tile([BL, 1], F32, name=f"{R}z_t")
    nc.scalar.activation(p_t, lg_sb, mybir.ActivationFunctionType.Exp,
                         bias=negmax, scale=1.0, accum_out=z_t)
    mf = gp.tile([BL, E], F32, name=f"{R}mf")
    nc.vector.tensor_copy(mf, mk_sb)          # int32 -> f32 cast
    g_t = gp.tile([BL, E], F32, name=f"{R}g_t")
    nc.vector.tensor_tensor(g_t, p_t, mf, mybir.AluOpType.mult)

    max8 = gp.tile([BL, 8], F32, name=f"{R}max8")
    idx8 = gp.tile([BL, 8], U32, name=f"{R}idx8")
    nc.vector.max(max8, g_t)
    nc.vector.max_index(idx8, max8, g_t)

    s0 = gp.tile([BL, 1], F32, name=f"{R}s0")
    nc.vector.tensor_tensor(s0, max8[:, 0:1], max8[:, 1:2], mybir.AluOpType.add)
    s1 = gp.tile([BL, 1], F32, name=f"{R}s1")
    nc.vector.scalar_tensor_tensor(s1, z_t, EPS, s0,
                                   mybir.AluOpType.mult, mybir.AluOpType.add)
    r_t = gp.tile([BL, 1], F32, name=f"{R}r_t")
    nc.vector.reciprocal(r_t, s1)
    gpair = gp.tile([BL, 2], F32, name=f"{R}gpair")
    nc.vector.tensor_tensor(gpair[:, 0:1], max8[:, 0:1], r_t, mybir.AluOpType.mult)
    nc.vector.tensor_tensor(gpair[:, 1:2], max8[:, 1:2], r_t, mybir.AluOpType.mult)

    # expert index -> element offset into a [128, E*D] weight tile
    off8 = gp.tile([BL, 8], U32, name=f"{R}off8")
    nc.vector.tensor_scalar(off8, idx8, 9, None, mybir.AluOpType.logical_shift_left)

    # rearrange per-sample scalars to a single partition-0 row:
    # row[0, 2b+i] = value(sample b, expert slot i)
    grow = gp.tile([1, 2 * BL], F32, name=f"{R}grow")
    orow = gp.tile([1, 2 * BL], U32, name=f"{R}orow")
    nc.gpsimd.dma_start(grow, gpair)
    nc.gpsimd.dma_start(orow, off8[:, 0:TOPK])

    # gates broadcast to all 128 partitions (per-partition scalar operand)
    gbc = gp.tile([128, 2 * BL], F32, name=f"{R}gbc")
    nc.gpsimd.partition_broadcast(gbc, grow)

    # weight offsets into PE registers (one multi-value reg load)
    _, offs = nc.values_load_multi_w_load_instructions(
        orow[0:1, :].bitcast(I32), engines=(PE,),
        min_val=0, max_val=(E - 1) * D, skip_runtime_bounds_check=True)

    # ---- pre-scale x by renormalized gates (DVE), all 32 copies upfront
    xs_t = {}
    for b in range(BL):
        ch = x_t[b // XCH]
        src = ch[:, (b % XCH) * KT * L:(b % XCH + 1) * KT * L]
        for i in range(TOPK):
            xs = xsp.tile([128, KT * L], BF16, tag="xs", name=f"{R}xs{b}_{i}")
            nc.vector.tensor_scalar_mul(xs, src, gbc[:, 2 * b + i:2 * b + i + 1])
            xs_t[(b, i)] = xs

    # ---- main matmul stream: 2 waves x 8 samples, k-major inside a wave
    for wave in range(BL // WAVE):
        psums = []
        for j in range(WAVE):
            psums.append(pp.tile([L, D], F32, tag="ps", name=f"{R}ps{wave}_{j}"))
        for k in range(KT):
            for j in range(WAVE):
                b = wave * WAVE + j
                for i in range(TOPK):
                    nc.tensor.matmul(
                        psums[j],
                        xs_t[(b, i)][:, k * L:(k + 1) * L],
                        w_t[k][:, bass.ds(offs[2 * b + i], D)],
                        start=(k == 0 and i == 0),
                        stop=(k == KT - 1 and i == TOPK - 1),
                    )
        for j in range(WAVE):
            b = wave * WAVE + j
            ot = op.tile([L, D], BF16, tag="ot", name=f"{R}ot{b}")
            nc.vector.tensor_copy(ot, psums[j])     # PSUM f32 -> SBUF bf16
            nc.scalar.dma_start(out[b], ot)         # ACT HWDGE ring


def _build_nc(repeats=1):
    nc = bacc.Bacc("TRN2", target_bir_lowering=False)

    xh = nc.declare_dram_parameter("xh", [128, BL * KT * L], BF16, isOutput=False)
    wh = nc.declare_dram_parameter("wh", [KT, 128, E * D], BF16, isOutput=False)
    lg = nc.declare_dram_parameter("lg", [BL, E], F32, isOutput=False)
    mk = nc.declare_dram_parameter("mk", [BL, E], I32, isOutput=False)
    out = nc.declare_dram_parameter("out", [BL, L, D], BF16, isOutput=True)

    with tile.TileContext(nc) as tc, ExitStack() as ctx:
        for rep in range(repeats):
            R = f"r{rep}_" if repeats > 1 else ""
            with ExitStack() as rctx:
                _emit_body(nc, tc, rctx, xh, wh, lg, mk, out, R=R)

    nc.compile()
    return nc


def get_nc(repeats=1):
    key = ("nc", repeats)
    if key not in _NC_CACHE:
        _NC_CACHE[key] = _build_nc(repeats)
    return _NC_CACHE[key]


def _prep_w(W, b):
    """-> [KT, 128, E*D] bf16: wh[k, p, e, d] = Wt_pad[e, 128k+p, d] where
    Wt_pad = [W_e^T (900 rows); bias_e (row 900); zeros (rows 901..1023)]."""
    wt = np.zeros((E, FP, D), np.float32)
    wt[:, :F, :] = np.asarray(W, np.float32).transpose(0, 2, 1)
    wt[:, F, :] = np.asarray(b, np.float32)
    wh = wt.reshape(E, KT, 128, D).transpose(1, 2, 0, 3).reshape(KT, 128, E * D)
    return np.ascontiguousarray(wh).astype(_BF)


def _prep_x(x):
    """-> [128, B, KT*L] bf16: xh[p, b, k*L+l] = xt_pad[b, 128k+p, l] where
    xt_pad = [x_b^T (900 rows); ones (row 900); zeros]."""
    x = np.asarray(x, np.float32).reshape(B, L, F)
    xt = np.zeros((B, FP, L), np.float32)
    xt[:, :F, :] = x.transpose(0, 2, 1)
    xt[:, F, :] = 1.0
    xh = xt.reshape(B, KT, 128, L).transpose(2, 0, 1, 3).reshape(128, B, KT * L)
    return np.ascontiguousarray(xh).astype(_BF)


LAST_RESULT = None


def kernel(cycle_curve_data, logits, moe_masks, W, b):
    global LAST_RESULT
    nc = get_nc()

    wh = _prep_w(W, b)
    xh = _prep_x(cycle_curve_data)
    lg = np.ascontiguousarray(np.asarray(logits, np.float32))
    mk = np.ascontiguousarray(np.asarray(moe_masks, np.int32))

    in_maps = []
    for c in range(NCORES):
        s = slice(c * BL, (c + 1) * BL)
        in_maps.append({
            "xh": np.ascontiguousarray(xh[:, s].reshape(128, BL * KT * L)),
            "wh": wh,
            "lg": np.ascontiguousarray(lg[s]),
            "mk": np.ascontiguousarray(mk[s]),
        })

    res = run_bass_kernel_spmd(nc, in_maps, core_ids=list(range(NCORES)))
    LAST_RESULT = res
    outs = [np.asarray(r["out"]) for r in res.results]
    return np.concatenate(outs, axis=0)
